# revision 55
# baseline (speedup 1.0000x reference)
"""Trainium2 kernel for the quantum-autoencoder forward pass (nn_AutoEncoder).

Math: the circuit uses only RX and CNOT gates on 8 data qubits (the 2 extra
trash-reference wires and the SWAP-test aux wire stay |0> until measurement).
Conjugating by H^x8 turns every RX into a diagonal RZ and every CNOT into a
basis permutation, so in the X-basis the state is always uniform-magnitude:
psi(x) = (1/16) e^{i theta(x)}, theta(x) = sum_g (t_g/2)(2<m_g,x> - 1) with
GF(2)^8 masks m_g evolved through the CNOT network.

The SWAP test gives p1 = (1 - P00)/2 with P00 = prob(trash wires 6,7 = |00>),
and in the X-frame P00 = (1/4)[1 + sum_{e in {e6,e7,e6^e7}} (1/256) *
sum_x cos(D_e(x))], D_e(x) = sum_{g:<m_g,e>=1} t_g (1 - 2<m_g,x>).

Flattened: p1[b] = 3/8 - (1/2048) * sum_{j<768} cos((A @ f_b)_j + (Pw @ w)_j)
with constant sign matrices A (768x8), Pw (768x32) from the circuit wiring.

Structure exploited on device: up to global row sign (cos is even) the 768
rows of [A|Pw] collapse to 384 distinct rows, each with multiplicity 2, and
those share only 12 distinct A-patterns. With U_k = (A_dist f)_k and
ctil_r = (Pw_dist w)_r:
    sum_j cos(D_j) = 2 * sum_k [ C_k cos(U_k) - S_k sin(U_k) ],
    C_k = sum_{r in grp k} cos(ctil_r),  S_k = sum_{r in grp k} sin(ctil_r)
so per batch row only 24 table lookups (12 sin + 12 cos) are needed; the
C/S weights are computed once per call from the 32 weights.

sin/cos evaluation: the scalar-engine Sin table is only valid on [-pi, pi],
so U is computed in *turns* (A_dist scaled by 1/2pi; cos columns get +0.25
turn) and range-reduced with the fp32 magic-number rounding trick:
t = V + 1.5*2^23 rounds V to the nearest integer k in the upper bits;
mr = (t - M) - V = k - V in [-0.5, 0.5]; sin(2pi V) = sin(-2pi * mr).

Device layout (per core, 512 batch rows, pure data parallel on 8 cores):
batch lives on the FREE axis, the 24 sin/cos terms on partitions. One K=9
matmul produces V^T [24, 512] for the whole shard, two DVE ops range-reduce,
one Sin activation evaluates all terms, and the weighted sum over the 24
terms is a K=24 matmul against the [-S | C] column -> [1, 512] results.
"""

import math
from contextlib import ExitStack

import numpy as np

import concourse.bass as bass
import concourse.tile as tile
from concourse import bacc, mybir
from concourse.bass_utils import run_bass_kernel_spmd

N_QUBITS = 8
DEPTH = 4
NW = DEPTH * N_QUBITS             # 32 weight angles
BATCH = 4096
N_CORES = 8
SHARD = BATCH // N_CORES          # 512 rows per core
P = 128                           # SBUF partitions
GROUPS = SHARD // P               # 4 batch groups of 128 per core
F32 = mybir.dt.float32
MAGIC = float(1.5 * 2**23)        # fp32 round-to-nearest-integer constant
TWO_PI_GUARD = 2.0 * math.pi * (1.0 - 2.0**-21)  # keep sin arg inside (-pi, pi)


def _build_raw_tables():
    """Phase-tracking masks for the fixed circuit -> sign matrices A, Pw."""
    gates = []  # [mask, ('f'|'w', index)]
    for w in range(N_QUBITS):
        gates.append([1 << w, ("f", w)])
    for l in range(DEPTH):
        for w in range(N_QUBITS):
            gates.append([1 << w, ("w", l * N_QUBITS + w)])
        for w in range(N_QUBITS):
            # original CNOT(ctrl=w, tgt=w+1) -> X-frame ctrl=w+1, tgt=w:
            # masks with bit w set get bit (w+1)%8 flipped
            t, c = w, (w + 1) % N_QUBITS
            for g in gates:
                if g[0] & (1 << t):
                    g[0] ^= 1 << c
    par = np.array([bin(i).count("1") & 1 for i in range(256)], np.int64)
    variants = [1 << 6, 1 << 7, (1 << 6) | (1 << 7)]
    A = np.zeros((3 * 256, N_QUBITS), np.float64)
    Pw = np.zeros((3 * 256, NW), np.float64)
    x = np.arange(256)
    for vi, e in enumerate(variants):
        rows = slice(vi * 256, (vi + 1) * 256)
        for m, (kind, idx) in gates:
            if par[m & e]:
                sigma = 1.0 - 2.0 * par[m & x]
                if kind == "f":
                    A[rows, idx] += sigma
                else:
                    Pw[rows, idx] += sigma
    return A, Pw


def _build_tables():
    A, Pw = _build_raw_tables()
    AB = np.concatenate([A, Pw], axis=1)  # (768, 40)
    # canonicalize row sign by leading nonzero (always in the A part)
    canon = []
    for r in AB:
        nz = np.nonzero(r)[0]
        s = 1.0 if r[nz[0]] > 0 else -1.0
        canon.append(tuple((s * r).tolist()))
    uniq = {}
    for c in canon:
        uniq[c] = uniq.get(c, 0) + 1
    assert len(uniq) == 384 and all(v == 2 for v in uniq.values())
    rows = np.array(list(uniq.keys()))          # (384, 40)
    a_rows = rows[:, :N_QUBITS]                 # (384, 8)
    pw_rows = rows[:, N_QUBITS:]                # (384, 32)
    a_uniq = {}
    for ar in map(tuple, a_rows):
        if ar not in a_uniq:
            a_uniq[ar] = len(a_uniq)
    K = len(a_uniq)
    assert K == 12
    grp = np.array([a_uniq[tuple(ar)] for ar in a_rows])  # (384,)
    a_dist = np.array(list(a_uniq.keys()))                # (12, 8)

    # AD2 (9, 25): contraction rows = 8 feature rows + 1 ones row.
    # cols 0:12 -> U_k in turns, 12:24 -> U_k + 0.25 turns, 24 -> constant
    # 0.25 turns (A=0), which makes sv row 24 == 1.0 and lets the final
    # affine (3/8 bias) ride the weighted-sum matmul.
    NT = 2 * K + 1
    ad2 = np.zeros((N_QUBITS + 1, NT), np.float64)
    ad2[:N_QUBITS, :K] = a_dist.T / (2 * math.pi)
    ad2[:N_QUBITS, K : 2 * K] = a_dist.T / (2 * math.pi)
    ad2[N_QUBITS, K:] = 0.25
    # AD4 (40, 128): block-diagonal over the 4 batch groups.  The transposed
    # fw tile ftp is [40, 128] (rows 10g:10g+10 = group g's 8 features, the
    # ones column, and the weights column); one K=40 matmul against AD4
    # yields V [128, 128] with group g's 25 terms at partitions 32g:32g+25
    # (pad rows read zero coefficients, so they come out 0.0, no uninit).
    ad4 = np.zeros((40, P), np.float64)
    for g in range(4):
        ad4[10 * g : 10 * g + N_QUBITS + 1, 32 * g : 32 * g + NT] = ad2
    # PWD (32, 384): ctil_r = (PWD.T @ w)_r in radians
    pwd = pw_rows.T
    # GM3 (128, 6*25): aggregation weights producing the final weight column
    # wv = [S/1024 | -C/1024 | 3/8] so that p1 = wv . sv directly.
    # Six K=128 matmuls accumulate wv[0:24]; matmul j uses lhsT =
    # GM3[:, 25j:25j+25] with rhs = sinc col j (j<3) / cosc col j-3.
    gm3 = np.zeros((P, 6 * NT), np.float64)
    for r in range(384):
        cc, p, k = r // P, r % P, grp[r]
        gm3[p, NT * cc + k] = 1.0 / 1024.0             # sin chunk -> +S_k/1024
        gm3[p, NT * (3 + cc) + K + k] = -1.0 / 1024.0  # cos chunk -> -C_k/1024
    # TE (1, 25): 3/8 at col 24, added to wv via a K=1 matmul against one1
    te = np.zeros((1, NT), np.float64)
    te[0, 2 * K] = 3.0 / 8.0
    return (
        ad4.astype(np.float32),
        np.ascontiguousarray(pwd.astype(np.float32)),
        gm3.astype(np.float32),
        te.astype(np.float32),
        K,
    )


_AD4, _PWD, _GM3, _TE, _K = _build_tables()
_NT = 2 * _K + 1


def _packed_consts():
    """Two constant blocks: pk1 [32, 384+25] = PWD | TE (for the ctil/wv
    setup), pk2 [128, 150] = GM3.  AD4 ships inside the fw input instead
    (cols 0:40 of the host-assembled [128, 80] image) so the V matmul is
    gated only by the single input DMA."""
    pk1 = np.zeros((NW, 3 * P + _NT), np.float32)
    pk1[:NW, : 3 * P] = _PWD
    pk1[:1, 3 * P :] = _TE
    return np.ascontiguousarray(pk1), np.ascontiguousarray(_GM3)


_PK1, _PK2 = _packed_consts()
_FWROWS = 40            # contraction rows (4 groups x 10 fields)
_FWCOLS = 2 * P + 1     # AD4 block | transposed feature block | weights col


def _host_fw_image(features: np.ndarray, weights: np.ndarray) -> np.ndarray:
    """Per-core [40, 257] SBUF image, already in matmul orientation:
    cols 0:128 = AD4 (lhsT), cols 128:256 = transposed feature blocks
    (row 10g+w = feature w of group g; w=8 row is all-ones), col 256 =
    the 32 weights on partitions 0:32."""
    feats = features.reshape(N_CORES, GROUPS, P, N_QUBITS)
    img = np.zeros((N_CORES, _FWROWS, _FWCOLS), np.float32)
    img[:, :, :P] = _AD4[None]
    for g in range(GROUPS):
        r = 10 * g
        img[:, r : r + N_QUBITS, P : 2 * P] = feats[:, g].transpose(0, 2, 1)
        img[:, r + N_QUBITS, P : 2 * P] = 1.0
    img[:, :NW, 2 * P] = weights.reshape(NW)[None, :]
    return img

_CACHE = {}


def _build_nc():
    NT = _NT  # 25: 12 sin + 12 cos + 1 const-one term
    FW = N_QUBITS + 2  # 10 cols: 8 features, all-ones bias col, weights col
    nc = bacc.Bacc(
        "TRN2",
        target_bir_lowering=False,
        debug=False,
        num_devices=N_CORES,
    )
    # fw: host-assembled [40, 257] SBUF image (see _host_fw_image).
    # Declared float32r end-to-end so the DMA itself is a valid producer
    # for the FP32R matmul (np-side it is plain float32 bits).
    fw = nc.dram_tensor(
        "fw", [_FWROWS, _FWCOLS], mybir.dt.float32r, kind="ExternalInput"
    )
    out = nc.dram_tensor("out", [GROUPS, P], F32, kind="ExternalOutput")
    pk1_d = nc.inline_tensor(_PK1, name="tabPK1")  # (32, 409) PWD|TE
    pk2_d = nc.inline_tensor(_PK2, name="tabPK2")  # (128, 150) GM3

    SIN = mybir.ActivationFunctionType.Sin
    SUB = mybir.AluOpType.subtract
    F32R = mybir.dt.float32r

    with tile.TileContext(nc) as tc, ExitStack() as ctx:
        const = ctx.enter_context(tc.tile_pool(name="const", bufs=1))
        work = ctx.enter_context(tc.tile_pool(name="work", bufs=2))
        sps = ctx.enter_context(tc.tile_pool(name="spsum", bufs=1, space="PSUM"))
        vps = ctx.enter_context(tc.tile_pool(name="vpsum", bufs=1, space="PSUM"))

        # dummy Sin first: triggers the ACT table load at t=0 so it overlaps
        # the input DMAs instead of sitting on the critical path
        one1 = const.tile([1, 1], F32)
        nc.gpsimd.memset(one1[:], 1.0)
        dummy = const.tile([1, 1], F32)
        nc.scalar.activation(dummy[:], one1[:], SIN, bias=one1[:], scale=0.0)

        # pk1 (PWD/TE) rides the Pool/SWDGE path, in parallel with the
        # HWDGE queue that carries fw (critical) and pk2 (GM3, needed last)
        pk1_s = const.tile([NW, _PK1.shape[1]], F32)
        nc.gpsimd.dma_start(pk1_s[:], pk1_d.ap()[:])
        # the input lands straight in an f32r tile in matmul orientation —
        # no on-chip transpose or conversion copy at all
        f_s = const.tile([_FWROWS, _FWCOLS], F32R)
        nc.sync.dma_start(f_s[:], fw.ap()[:])
        pk2_s = const.tile([P, _PK2.shape[1]], F32)
        nc.sync.dma_start(pk2_s[:], pk2_d.ap()[:])
        pwd_s = pk1_s[:NW, : 3 * P]
        te_s = pk1_s[:1, 3 * P :]
        w_s = f_s[:NW, 2 * P : 2 * P + 1].bitcast(F32)

        zeros = const.tile([P, 1], F32)
        nc.gpsimd.memset(zeros[:], 0.0)
        halfpi = const.tile([P, 1], F32)
        nc.gpsimd.memset(halfpi[:], math.pi / 2)

        # ctil matmuls early on PE (they only need pk1 + the weights column)
        ct_p = sps.tile([P, 3], F32, tag="setup")
        for cc in range(3):
            nc.tensor.matmul(
                ct_p[:, cc : cc + 1], pwd_s[:, P * cc : P * (cc + 1)], w_s,
                start=True, stop=True,
            )
        # V [128, 128]: group g's 25 terms at partitions 32g:32g+25, batch
        # within group on the free axis; pad rows compute to exactly 0
        v_p = vps.tile([P, P], F32, tag="v")
        nc.tensor.matmul(
            v_p[:], f_s[:, :P], f_s[:, P : 2 * P], start=True, stop=True
        )
        t_s = work.tile([P, P], F32, tag="t")
        nc.vector.tensor_scalar_add(t_s[:], v_p[:], MAGIC)
        mr_s = work.tile([P, P], F32, tag="mr")
        nc.vector.scalar_tensor_tensor(
            mr_s[:], t_s[:], MAGIC, v_p[:], op0=SUB, op1=SUB
        )

        # ---- weight setup (off critical path; ACT before the big Sin)
        sinc = const.tile([P, 3], F32)
        nc.scalar.activation(sinc[:], ct_p[:], SIN, bias=zeros[:], scale=1.0)
        cosc = const.tile([P, 3], F32)
        nc.scalar.activation(cosc[:], ct_p[:], SIN, bias=halfpi[:], scale=-1.0)
        # wv4 [128, 4]: col g = [S/1024 | -C/1024 | 3/8] at rows 32g:32g+25,
        # zero elsewhere (memset; the matmuls overwrite their block)
        wv_p = sps.tile([P, GROUPS], F32, tag="setup2")
        nc.vector.memset(wv_p[:], 0.0)
        for g in range(GROUPS):
            blk = wv_p[32 * g : 32 * g + NT, g : g + 1]
            tp = (0, 32 * g)  # explicit: base_partition() rejects 96
            nc.tensor.matmul(
                blk, te_s, one1[:], start=True, stop=False, tile_position=tp
            )
            for j in range(6):
                sc, cc = (sinc, j) if j < 3 else (cosc, j - 3)
                nc.tensor.matmul(
                    blk, pk2_s[:, NT * j : NT * (j + 1)], sc[:, cc : cc + 1],
                    start=False, stop=(j == 5), tile_position=tp,
                )
        wv4 = const.tile([P, GROUPS], F32R)
        nc.vector.tensor_copy(wv4[:], wv_p[:])

        # ---- tail of the main chain
        sv_s = work.tile([P, P], F32R, tag="sv")
        nc.scalar.activation(
            sv_s[:], mr_s[:], SIN, bias=zeros[:], scale=-TWO_PI_GUARD
        )
        p_p = vps.tile([GROUPS, P], F32, tag="p")
        nc.tensor.matmul(p_p[:], wv4[:], sv_s[:], start=True, stop=True)
        res = const.tile([GROUPS, P], F32)
        nc.vector.tensor_copy(res[:], p_p[:])
        nc.sync.dma_start(out.ap()[:], res[:])

    nc.compile()
    return nc


def get_nc():
    if "nc" not in _CACHE:
        _CACHE["nc"] = _build_nc()
    return _CACHE["nc"]


def kernel(features: np.ndarray, weights: np.ndarray, **run_kwargs) -> np.ndarray:
    nc = get_nc()
    fw = _host_fw_image(
        np.ascontiguousarray(features, np.float32),
        np.ascontiguousarray(weights, np.float32),
    )
    in_maps = [{"fw": fw[i]} for i in range(N_CORES)]
    last_err = None
    for attempt in range(3):
        try:
            r = run_bass_kernel_spmd(
                nc, in_maps, core_ids=list(range(N_CORES)), **run_kwargs
            )
            break
        except Exception as e:  # transient device-unrecoverable states
            last_err = e
            if attempt == 2:
                raise
            import time

            time.sleep(45)
    out = np.concatenate(
        [np.asarray(r.results[i]["out"]).reshape(SHARD) for i in range(N_CORES)]
    )
    if run_kwargs:
        return out.astype(np.float32), r
    return out.astype(np.float32)


# revision 64
# speedup vs baseline: 1.0386x; 1.0386x over previous
"""Trainium2 kernel for the quantum-autoencoder forward pass (nn_AutoEncoder).

Math: the circuit uses only RX and CNOT gates on 8 data qubits (the 2 extra
trash-reference wires and the SWAP-test aux wire stay |0> until measurement).
Conjugating by H^x8 turns every RX into a diagonal RZ and every CNOT into a
basis permutation, so in the X-basis the state is always uniform-magnitude:
psi(x) = (1/16) e^{i theta(x)}, theta(x) = sum_g (t_g/2)(2<m_g,x> - 1) with
GF(2)^8 masks m_g evolved through the CNOT network.

The SWAP test gives p1 = (1 - P00)/2 with P00 = prob(trash wires 6,7 = |00>),
and in the X-frame P00 = (1/4)[1 + sum_{e in {e6,e7,e6^e7}} (1/256) *
sum_x cos(D_e(x))], D_e(x) = sum_{g:<m_g,e>=1} t_g (1 - 2<m_g,x>).

Flattened: p1[b] = 3/8 - (1/2048) * sum_{j<768} cos((A @ f_b)_j + (Pw @ w)_j)
with constant sign matrices A (768x8), Pw (768x32) from the circuit wiring.

Structure exploited on device: up to global row sign (cos is even) the 768
rows of [A|Pw] collapse to 384 distinct rows, each with multiplicity 2, and
those share only 12 distinct A-patterns. With U_k = (A_dist f)_k and
ctil_r = (Pw_dist w)_r:
    sum_j cos(D_j) = 2 * sum_k [ C_k cos(U_k) - S_k sin(U_k) ],
    C_k = sum_{r in grp k} cos(ctil_r),  S_k = sum_{r in grp k} sin(ctil_r)
so per batch row only 24 table lookups (12 sin + 12 cos) are needed; the
C/S weights are computed once per call from the 32 weights.

sin/cos evaluation: the scalar-engine Sin table is only valid on [-pi, pi],
so U is computed in *turns* (A_dist scaled by 1/2pi; cos columns get +0.25
turn) and range-reduced with the fp32 magic-number rounding trick:
t = V + 1.5*2^23 rounds V to the nearest integer k in the upper bits;
mr = (t - M) - V = k - V in [-0.5, 0.5]; sin(2pi V) = sin(-2pi * mr).

Device layout (per core, 512 batch rows, pure data parallel on 8 cores):
batch lives on the FREE axis, the 24 sin/cos terms on partitions. One K=9
matmul produces V^T [24, 512] for the whole shard, two DVE ops range-reduce,
one Sin activation evaluates all terms, and the weighted sum over the 24
terms is a K=24 matmul against the [-S | C] column -> [1, 512] results.
"""

import math
from contextlib import ExitStack

import numpy as np

import concourse.bass as bass
import concourse.tile as tile
from concourse import bacc, mybir
from concourse.bass_utils import run_bass_kernel_spmd

N_QUBITS = 8
DEPTH = 4
NW = DEPTH * N_QUBITS             # 32 weight angles
BATCH = 4096
N_CORES = 8
SHARD = BATCH // N_CORES          # 512 rows per core
P = 128                           # SBUF partitions
GROUPS = SHARD // P               # 4 batch groups of 128 per core
F32 = mybir.dt.float32
MAGIC = float(1.5 * 2**23)        # fp32 round-to-nearest-integer constant
TWO_PI_GUARD = 2.0 * math.pi * (1.0 - 2.0**-21)  # keep sin arg inside (-pi, pi)


def _build_raw_tables():
    """Phase-tracking masks for the fixed circuit -> sign matrices A, Pw."""
    gates = []  # [mask, ('f'|'w', index)]
    for w in range(N_QUBITS):
        gates.append([1 << w, ("f", w)])
    for l in range(DEPTH):
        for w in range(N_QUBITS):
            gates.append([1 << w, ("w", l * N_QUBITS + w)])
        for w in range(N_QUBITS):
            # original CNOT(ctrl=w, tgt=w+1) -> X-frame ctrl=w+1, tgt=w:
            # masks with bit w set get bit (w+1)%8 flipped
            t, c = w, (w + 1) % N_QUBITS
            for g in gates:
                if g[0] & (1 << t):
                    g[0] ^= 1 << c
    par = np.array([bin(i).count("1") & 1 for i in range(256)], np.int64)
    variants = [1 << 6, 1 << 7, (1 << 6) | (1 << 7)]
    A = np.zeros((3 * 256, N_QUBITS), np.float64)
    Pw = np.zeros((3 * 256, NW), np.float64)
    x = np.arange(256)
    for vi, e in enumerate(variants):
        rows = slice(vi * 256, (vi + 1) * 256)
        for m, (kind, idx) in gates:
            if par[m & e]:
                sigma = 1.0 - 2.0 * par[m & x]
                if kind == "f":
                    A[rows, idx] += sigma
                else:
                    Pw[rows, idx] += sigma
    return A, Pw


def _build_tables():
    A, Pw = _build_raw_tables()
    AB = np.concatenate([A, Pw], axis=1)  # (768, 40)
    # canonicalize row sign by leading nonzero (always in the A part)
    canon = []
    for r in AB:
        nz = np.nonzero(r)[0]
        s = 1.0 if r[nz[0]] > 0 else -1.0
        canon.append(tuple((s * r).tolist()))
    uniq = {}
    for c in canon:
        uniq[c] = uniq.get(c, 0) + 1
    assert len(uniq) == 384 and all(v == 2 for v in uniq.values())
    rows = np.array(list(uniq.keys()))          # (384, 40)
    a_rows = rows[:, :N_QUBITS]                 # (384, 8)
    pw_rows = rows[:, N_QUBITS:]                # (384, 32)
    a_uniq = {}
    for ar in map(tuple, a_rows):
        if ar not in a_uniq:
            a_uniq[ar] = len(a_uniq)
    K = len(a_uniq)
    assert K == 12
    grp = np.array([a_uniq[tuple(ar)] for ar in a_rows])  # (384,)
    a_dist = np.array(list(a_uniq.keys()))                # (12, 8)

    # AD2 (9, 25): contraction rows = 8 feature rows + 1 ones row.
    # cols 0:12 -> U_k in turns, 12:24 -> U_k + 0.25 turns, 24 -> constant
    # 0.25 turns (A=0), which makes sv row 24 == 1.0 and lets the final
    # affine (3/8 bias) ride the weighted-sum matmul.
    NT = 2 * K + 1
    ad2 = np.zeros((N_QUBITS + 1, NT), np.float64)
    ad2[:N_QUBITS, :K] = a_dist.T / (2 * math.pi)
    ad2[:N_QUBITS, K : 2 * K] = a_dist.T / (2 * math.pi)
    ad2[N_QUBITS, K:] = 0.25
    # AD4 (40, 128): block-diagonal over the 4 batch groups.  The transposed
    # fw tile ftp is [40, 128] (rows 10g:10g+10 = group g's 8 features, the
    # ones column, and the weights column); one K=40 matmul against AD4
    # yields V [128, 128] with group g's 25 terms at partitions 32g:32g+25
    # (pad rows read zero coefficients, so they come out 0.0, no uninit).
    ad4 = np.zeros((40, P), np.float64)
    for g in range(4):
        ad4[10 * g : 10 * g + N_QUBITS + 1, 32 * g : 32 * g + NT] = ad2
    # PWD2 (33, 768): chunk j<3 gives ctil values 128j:128j+128; chunk
    # j>=3 gives pi/2 - ctil for the same values (row 32 pairs with the
    # constant-one entry of the weights column, contributing the pi/2)
    pwd = pw_rows.T  # (32, 384)
    pwd2 = np.zeros((NW + 1, 6 * P), np.float64)
    pwd2[:NW, : 3 * P] = pwd
    pwd2[:NW, 3 * P :] = -pwd
    pwd2[NW, 3 * P :] = math.pi / 2
    # GM3 (128, 6*25): aggregation weights producing the final weight column
    # wv = [S/1024 | -C/1024 | 3/8] so that p1 = wv . sv directly.
    # Six K=128 matmuls accumulate wv[0:24]; matmul j uses lhsT =
    # GM3[:, 25j:25j+25] with rhs = sinc col j (j<3) / cosc col j-3.
    gm3 = np.zeros((P, 6 * NT), np.float64)
    for r in range(384):
        cc, p, k = r // P, r % P, grp[r]
        gm3[p, NT * cc + k] = 1.0 / 1024.0             # sin chunk -> +S_k/1024
        gm3[p, NT * (3 + cc) + K + k] = -1.0 / 1024.0  # cos chunk -> -C_k/1024
    # TE (1, 25): 3/8 at col 24, added to wv via a K=1 matmul against one1
    te = np.zeros((1, NT), np.float64)
    te[0, 2 * K] = 3.0 / 8.0
    return (
        ad4.astype(np.float32),
        np.ascontiguousarray(pwd2.astype(np.float32)),
        gm3.astype(np.float32),
        te.astype(np.float32),
        K,
    )


_AD4, _PWD2, _GM3, _TE, _K = _build_tables()
_NT = 2 * _K + 1


def _packed_consts():
    """Two constant blocks: pk1 [33, 768+25] = PWD2 | TE (for the ctil/wv
    setup), pk2 [128, 150] = GM3.  AD4 ships inside the fw input instead
    so the V matmul is gated only by the single input DMA."""
    pk1 = np.zeros((NW + 1, 6 * P + _NT), np.float32)
    pk1[: NW + 1, : 6 * P] = _PWD2
    pk1[:1, 6 * P :] = _TE
    return np.ascontiguousarray(pk1), np.ascontiguousarray(_GM3)


_PK1, _PK2 = _packed_consts()
_FWROWS = 40            # contraction rows (4 groups x 10 fields)
_FWCOLS = 2 * P + 2     # AD4 block | transposed feature block | w col | one


def _host_fw_image(features: np.ndarray, weights: np.ndarray) -> np.ndarray:
    """Per-core [40, 258] SBUF image, already in matmul orientation:
    cols 0:128 = AD4 (lhsT), cols 128:256 = transposed feature blocks
    (row 10g+w = feature w of group g; w=8 row is all-ones), col 256 =
    the 32 weights on partitions 0:32 plus a 1.0 at partition 32 (pairs
    with PWD2's pi/2 row), col 257 = 1.0 at partition 0 (TE's rhs)."""
    feats = features.reshape(N_CORES, GROUPS, P, N_QUBITS)
    img = np.zeros((N_CORES, _FWROWS, _FWCOLS), np.float32)
    img[:, :, :P] = _AD4[None]
    for g in range(GROUPS):
        r = 10 * g
        img[:, r : r + N_QUBITS, P : 2 * P] = feats[:, g].transpose(0, 2, 1)
        img[:, r + N_QUBITS, P : 2 * P] = 1.0
    img[:, :NW, 2 * P] = weights.reshape(NW)[None, :]
    img[:, NW, 2 * P] = 1.0
    img[:, 0, 2 * P + 1] = 1.0
    return img

_CACHE = {}


def _build_nc():
    NT = _NT  # 25: 12 sin + 12 cos + 1 const-one term
    FW = N_QUBITS + 2  # 10 cols: 8 features, all-ones bias col, weights col
    nc = bacc.Bacc(
        "TRN2",
        target_bir_lowering=False,
        debug=False,
        num_devices=N_CORES,
    )
    # fw: host-assembled [40, 257] SBUF image (see _host_fw_image).
    # Declared float32r end-to-end so the DMA itself is a valid producer
    # for the FP32R matmul (np-side it is plain float32 bits).
    fw = nc.dram_tensor(
        "fw", [_FWROWS, _FWCOLS], mybir.dt.float32r, kind="ExternalInput"
    )
    out = nc.dram_tensor("out", [GROUPS, P], F32, kind="ExternalOutput")
    pk1_d = nc.inline_tensor(_PK1, name="tabPK1")  # (33, 793) PWD2|TE
    pk2_d = nc.inline_tensor(_PK2, name="tabPK2")  # (128, 150) GM3

    SIN = mybir.ActivationFunctionType.Sin
    SUB = mybir.AluOpType.subtract
    F32R = mybir.dt.float32r

    with tile.TileContext(nc) as tc, ExitStack() as ctx:
        const = ctx.enter_context(tc.tile_pool(name="const", bufs=1))
        work = ctx.enter_context(tc.tile_pool(name="work", bufs=2))
        sps = ctx.enter_context(tc.tile_pool(name="spsum", bufs=1, space="PSUM"))
        vps = ctx.enter_context(tc.tile_pool(name="vpsum", bufs=1, space="PSUM"))

        # dummy Sin first: triggers the ACT table load at t=0 so it overlaps
        # the input DMAs instead of sitting on the critical path
        onec = const.tile([1, 1], F32)
        nc.gpsimd.memset(onec[:], 1.0)
        dummy = const.tile([1, 1], F32)
        nc.scalar.activation(dummy[:], onec[:], SIN, bias=onec[:], scale=0.0)

        # pk1 (PWD2/TE) rides the Pool/SWDGE path, in parallel with the
        # HWDGE queue that carries fw (critical) and pk2 (GM3, needed last)
        pk1_s = const.tile([NW + 1, _PK1.shape[1]], F32)
        nc.gpsimd.dma_start(pk1_s[:], pk1_d.ap()[:])
        # the input lands straight in an f32r tile in matmul orientation —
        # no on-chip transpose or conversion copy at all
        f_s = const.tile([_FWROWS, _FWCOLS], F32R)
        nc.sync.dma_start(f_s[:], fw.ap()[:])
        pk2_s = const.tile([P, _PK2.shape[1]], F32)
        nc.sync.dma_start(pk2_s[:], pk2_d.ap()[:])
        pwd2_s = pk1_s[: NW + 1, : 6 * P]
        te_s = pk1_s[:1, 6 * P :]
        w2_s = f_s[: NW + 1, 2 * P : 2 * P + 1].bitcast(F32)
        one1 = f_s[:1, 2 * P + 1 : 2 * P + 2].bitcast(F32)

        zeros = const.tile([P, 1], F32)
        nc.gpsimd.memset(zeros[:], 0.0)

        # ctil matmuls early on PE (they only need pk1 + the weights column);
        # cols 0:3 accumulate ctil, cols 3:6 accumulate pi/2 - ctil so ONE
        # Sin activation yields both sin and cos of the weight angles
        ct_p = sps.tile([P, 6], F32, tag="setup")
        for j in range(6):
            nc.tensor.matmul(
                ct_p[:, j : j + 1], pwd2_s[:, P * j : P * (j + 1)], w2_s,
                start=True, stop=True,
            )
        # V [128, 128]: group g's 25 terms at partitions 32g:32g+25, batch
        # within group on the free axis; pad rows compute to exactly 0
        v_p = vps.tile([P, P], F32, tag="v")
        nc.tensor.matmul(
            v_p[:], f_s[:, :P], f_s[:, P : 2 * P], start=True, stop=True
        )
        t_s = work.tile([P, P], F32, tag="t")
        nc.vector.tensor_scalar_add(t_s[:], v_p[:], MAGIC)
        mr_s = work.tile([P, P], F32, tag="mr")
        nc.vector.scalar_tensor_tensor(
            mr_s[:], t_s[:], MAGIC, v_p[:], op0=SUB, op1=SUB
        )

        # ---- weight setup (off critical path; ACT before the big Sin)
        sc6 = const.tile([P, 6], F32)
        nc.scalar.activation(sc6[:], ct_p[:], SIN, bias=zeros[:], scale=1.0)
        # wv4 [128, 4]: col g = [S/1024 | -C/1024 | 3/8] at rows 32g:32g+25,
        # zero elsewhere (memset; the matmuls overwrite their block)
        wv_p = sps.tile([P, GROUPS], F32, tag="setup2")
        nc.vector.memset(wv_p[:], 0.0)
        for g in range(GROUPS):
            blk = wv_p[32 * g : 32 * g + NT, g : g + 1]
            tp = (0, 32 * g)  # explicit: base_partition() rejects 96
            nc.tensor.matmul(
                blk, te_s, one1, start=True, stop=False, tile_position=tp
            )
            for j in range(6):
                nc.tensor.matmul(
                    blk, pk2_s[:, NT * j : NT * (j + 1)], sc6[:, j : j + 1],
                    start=False, stop=(j == 5), tile_position=tp,
                )
        wv4 = const.tile([P, GROUPS], F32R)
        nc.vector.tensor_copy(wv4[:], wv_p[:])

        # ---- tail of the main chain
        sv_s = work.tile([P, P], F32R, tag="sv")
        nc.scalar.activation(
            sv_s[:], mr_s[:], SIN, bias=zeros[:], scale=-TWO_PI_GUARD
        )
        p_p = vps.tile([GROUPS, P], F32, tag="p")
        nc.tensor.matmul(p_p[:], wv4[:], sv_s[:], start=True, stop=True)
        res = const.tile([GROUPS, P], F32)
        nc.vector.tensor_copy(res[:], p_p[:])
        nc.sync.dma_start(out.ap()[:], res[:])

    nc.compile()
    return nc


def get_nc():
    if "nc" not in _CACHE:
        _CACHE["nc"] = _build_nc()
    return _CACHE["nc"]


def kernel(features: np.ndarray, weights: np.ndarray, **run_kwargs) -> np.ndarray:
    nc = get_nc()
    fw = _host_fw_image(
        np.ascontiguousarray(features, np.float32),
        np.ascontiguousarray(weights, np.float32),
    )
    in_maps = [{"fw": fw[i]} for i in range(N_CORES)]
    last_err = None
    for attempt in range(3):
        try:
            r = run_bass_kernel_spmd(
                nc, in_maps, core_ids=list(range(N_CORES)), **run_kwargs
            )
            break
        except Exception as e:  # transient device-unrecoverable states
            last_err = e
            if attempt == 2:
                raise
            import time

            time.sleep(45)
    out = np.concatenate(
        [np.asarray(r.results[i]["out"]).reshape(SHARD) for i in range(N_CORES)]
    )
    if run_kwargs:
        return out.astype(np.float32), r
    return out.astype(np.float32)
